# revision 1
# baseline (speedup 1.0000x reference)
"""Trainium2 Bass kernel for attention-based seq2seq GRU (nn_GRU).

Data-parallel over batch B=64 across 8 cores (8 lanes/core, no collectives).
Per core: device-side embedding gather, d-major GRU scans with bulk
x-precompute, bulk per-t attention (PE matvec scores), softmax via
ones-matmul column sums, fused output chain.
"""

import numpy as np

import concourse.bass as bass
import concourse.bacc as bacc
import concourse.mybir as mybir
import concourse.tile as tile
from concourse.bass import IndirectOffsetOnAxis
from concourse.bass_utils import run_bass_kernel_spmd
from concourse.masks import make_identity

F32 = mybir.dt.float32
I32 = mybir.dt.int32
AF = mybir.ActivationFunctionType

T, B, H, D2, BL, NCORE, VY = 128, 64, 256, 512, 8, 8, 12
TD = T - 1

_prog_cache = {}
last_results = None


def build_program():
    nc = bacc.Bacc(None, target_bir_lowering=False)

    def _w(name, shape):
        return nc.dram_tensor(name, list(shape), F32, kind="ExternalInput")

    tok = nc.dram_tensor("tok", [T, BL], I32, kind="ExternalInput")
    we = _w("we", [100000, H])
    wx_f = _w("wx_f", [128, 2, D2]); wh_f = _w("wh_f", [128, 2, D2])
    wxh_f = _w("wxh_f", [128, 2, H]); whh_f = _w("whh_f", [128, 2, H])
    wx_b = _w("wx_b", [128, 2, D2]); wh_b = _w("wh_b", [128, 2, D2])
    wxh_b = _w("wxh_b", [128, 2, H]); whh_b = _w("whh_b", [128, 2, H])
    wx_d = _w("wx_d", [128, 4, D2]); wh_d = _w("wh_d", [128, 2, D2])
    wxh_d = _w("wxh_d", [128, 4, H]); whh_d = _w("whh_d", [128, 2, H])
    wa_c = _w("wa_c", [128, 4, D2]); wa_h = _w("wa_h", [128, 2, D2])
    way = _w("way", [128, 4])
    wf_c = _w("wf_c", [128, 4, H]); wf_f = _w("wf_f", [128, 2, H])
    wf_h = _w("wf_h", [128, 2, H])
    wy = _w("wy", [128, 2, VY])
    b_f = _w("b_f", [1, D2]); bh_f = _w("bh_f", [1, H])
    b_b = _w("b_b", [1, D2]); bh_b = _w("bh_b", [1, H])
    b_d = _w("b_d", [1, D2]); bh_d = _w("bh_d", [1, H])
    ba = _w("ba", [1, D2]); bfu = _w("bfu", [1, H])
    by = _w("by", [1, VY])

    y_out = nc.dram_tensor("y", [VY, TD, BL], F32, kind="ExternalOutput")

    with tile.TileContext(nc) as tc:
        with tc.tile_pool(name="pers", bufs=1) as pers:
            def load(pool, t_dram, shape):
                tl = pool.tile(list(shape), F32, tag=t_dram.name + "_s")
                nc.sync.dma_start(out=tl[:], in_=t_dram[:])
                return tl

            swh_f = load(pers, wh_f, [128, 2, D2]); swhh_f = load(pers, whh_f, [128, 2, H])
            swh_b = load(pers, wh_b, [128, 2, D2]); swhh_b = load(pers, whh_b, [128, 2, H])
            swh_d = load(pers, wh_d, [128, 2, D2]); swhh_d = load(pers, whh_d, [128, 2, H])
            swa_c = load(pers, wa_c, [128, 4, D2]); swa_h = load(pers, wa_h, [128, 2, D2])
            sway = load(pers, way, [128, 4])
            swf_c = load(pers, wf_c, [128, 4, H]); swf_f = load(pers, wf_f, [128, 2, H])
            swf_h = load(pers, wf_h, [128, 2, H]); swy = load(pers, wy, [128, 2, VY])
            sb_d = load(pers, b_d, [1, D2]); sbh_d = load(pers, bh_d, [1, H])
            sba = load(pers, ba, [1, D2]); sbfu = load(pers, bfu, [1, H])
            sby = load(pers, by, [1, VY])

            ident = pers.tile([128, 128], F32, tag="ident")
            make_identity(nc, ident[:])
            ones_row = pers.tile([1, 128], F32, tag="ones_row")
            nc.vector.memset(ones_row[:], 1.0)
            ones3 = pers.tile([1, 64, BL], F32, tag="ones3")
            nc.vector.memset(ones3[:], 1.0)
            ones_col = pers.tile([128, 1], F32, tag="ones_col")
            nc.vector.memset(ones_col[:], 1.0)
            h0 = pers.tile([128, 2, BL], F32, tag="h0")
            nc.vector.memset(h0[:], 0.0)

            ctx_d = pers.tile([128, 4, T, BL], F32, tag="ctx_d")
            ctxT1 = pers.tile([128, BL, D2], F32, tag="ctxT1")

            def bias_mm(ps_slice, bias_ap, nt):
                nc.tensor.matmul(out=ps_slice, lhsT=bias_ap,
                                 rhs=ones3[:, 0:nt, :], start=False, stop=True)

            # ---- phase 1: gather + transpose + enc x-precompute ----
            with tc.tile_pool(name="enc", bufs=1) as enc:
                swx_f = load(enc, wx_f, [128, 2, D2]); swxh_f = load(enc, wxh_f, [128, 2, H])
                swx_b = load(enc, wx_b, [128, 2, D2]); swxh_b = load(enc, wxh_b, [128, 2, H])
                sb_f = load(enc, b_f, [1, D2]); sbh_f = load(enc, bh_f, [1, H])
                sb_b = load(enc, b_b, [1, D2]); sbh_b = load(enc, bh_b, [1, H])

                embT = enc.tile([128, 2, T, BL], F32, tag="embT")
                xf = enc.tile([128, 4, T, BL], F32, tag="xf")
                xhf = enc.tile([128, 2, T, BL], F32, tag="xhf")
                xb = enc.tile([128, 4, T, BL], F32, tag="xb")
                xhb = enc.tile([128, 2, T, BL], F32, tag="xhb")

                with tc.tile_pool(name="ps_g", bufs=2, space="PSUM") as psg:
                    # dummy transpose so PE observes the gpsimd identity
                    # semaphore before the real transposes (keeps each real
                    # transpose at a single sync wait — S3_LW slot limit)
                    pst0 = psg.tile([128, 128], F32, tag="tr")
                    nc.tensor.transpose(out=pst0[:], in_=ident[:], identity=ident[:])
                    for b in range(BL):
                        idx = enc.tile([128, 1], I32, tag=f"idx{b}")
                        nc.sync.dma_start(out=idx[:], in_=tok[:, b:b + 1])
                        embr = enc.tile([128, H], F32, tag=f"embr{b}")
                        nc.gpsimd.indirect_dma_start(
                            out=embr[:], out_offset=None, in_=we[:],
                            in_offset=IndirectOffsetOnAxis(ap=idx[:, :1], axis=0))
                        # bounce through DVE so the PE transpose has a single
                        # upstream semaphore (indirect DMA fans across queues)
                        embc = enc.tile([128, H], F32, tag=f"embc{b}")
                        nc.vector.tensor_copy(out=embc[:], in_=embr[:])
                        for k in range(2):
                            pst = psg.tile([128, 128], F32, tag="tr")
                            nc.tensor.transpose(out=pst[:], in_=embc[:, 128 * k:128 * (k + 1)],
                                                identity=ident[:])
                            nc.vector.tensor_copy(out=embT[:, k, :, b], in_=pst[:])

                    def xbulk(dst, wt, bias, mchunks):
                        for m in range(mchunks):
                            for nb in range(2):
                                ps = psg.tile([128, 64, BL], F32, tag="xb_ps")
                                tsl = slice(64 * nb, 64 * (nb + 1))
                                for k in range(2):
                                    nc.tensor.matmul(
                                        out=ps[:], lhsT=wt[:, k, 128 * m:128 * (m + 1)],
                                        rhs=embT[:, k, tsl, :], start=(k == 0), stop=False)
                                bias_mm(ps[:], bias[:, 128 * m:128 * (m + 1)], 64)
                                nc.vector.tensor_copy(out=dst[:, m, tsl, :], in_=ps[:])

                    xbulk(xf, swx_f, sb_f, 4)
                    xbulk(xhf, swxh_f, sbh_f, 2)
                    xbulk(xb, swx_b, sb_b, 4)
                    xbulk(xhb, swxh_b, sbh_b, 2)

                # ---- phase 2: encoder scans ----
                with tc.tile_pool(name="ps_scan", bufs=2, space="PSUM") as pss:
                    def gru_step(tag, pool, wh, whh, xsl, xhsl, hprev, hout_ap):
                        ps_rz = pss.tile([128, 4, BL], F32, tag=f"rz_{tag}")
                        for m in range(4):
                            for k in range(2):
                                nc.tensor.matmul(
                                    out=ps_rz[:, m, :],
                                    lhsT=wh[:, k, 128 * m:128 * (m + 1)],
                                    rhs=hprev[:, k, :], start=(k == 0), stop=(k == 1))
                        rz = pool.tile([128, 4, BL], F32, tag=f"rzs_{tag}")
                        nc.vector.tensor_add(out=rz[:], in0=ps_rz[:], in1=xsl)
                        rs = pool.tile([128, 4, BL], F32, tag=f"rs_{tag}")
                        nc.scalar.activation(out=rs[:], in_=rz[:], func=AF.Sigmoid)
                        rh = pool.tile([128, 2, BL], F32, tag=f"rh_{tag}")
                        nc.vector.tensor_mul(out=rh[:], in0=rs[:, 0:2, :], in1=hprev[:])
                        ps_hc = pss.tile([128, 2, BL], F32, tag=f"hc_{tag}")
                        for m in range(2):
                            for k in range(2):
                                nc.tensor.matmul(
                                    out=ps_hc[:, m, :],
                                    lhsT=whh[:, k, 128 * m:128 * (m + 1)],
                                    rhs=rh[:, k, :], start=(k == 0), stop=(k == 1))
                        hcp = pool.tile([128, 2, BL], F32, tag=f"hcp_{tag}")
                        nc.vector.tensor_add(out=hcp[:], in0=ps_hc[:], in1=xhsl)
                        hc = pool.tile([128, 2, BL], F32, tag=f"hcs_{tag}")
                        nc.scalar.activation(out=hc[:], in_=hcp[:], func=AF.Tanh)
                        tmp = pool.tile([128, 2, BL], F32, tag=f"tmp_{tag}")
                        nc.vector.tensor_sub(out=tmp[:], in0=hprev[:], in1=hc[:])
                        nc.vector.tensor_mul(out=tmp[:], in0=rs[:, 2:4, :], in1=tmp[:])
                        nc.vector.tensor_add(out=hout_ap, in0=hc[:], in1=tmp[:])

                    for t in range(T):
                        hp = h0[:] if t == 0 else ctx_d[:, 0:2, t - 1, :]
                        gru_step("f", enc, swh_f, swhh_f, xf[:, :, t, :],
                                 xhf[:, :, t, :], hp, ctx_d[:, 0:2, t, :])
                        tb = T - 1 - t
                        hpb = h0[:] if t == 0 else ctx_d[:, 2:4, tb + 1, :]
                        gru_step("b", enc, swh_b, swhh_b, xb[:, :, tb, :],
                                 xhb[:, :, tb, :], hpb, ctx_d[:, 2:4, tb, :])

            # ---- phase 3: ctxT1 + pctx ----
            with tc.tile_pool(name="mid", bufs=1) as mid:
                pctx = mid.tile([128, 4, T, BL], F32, tag="pctx")
                q = mid.tile([128, 4, TD, BL], F32, tag="q")
                hdT = mid.tile([128, 2, T, BL], F32, tag="hdT")
                scores = mid.tile([128, TD, BL], F32, tag="scores")

                with tc.tile_pool(name="ps_mid", bufs=4, space="PSUM") as psm:
                    for b in range(BL):
                        for k in range(4):
                            pst = psm.tile([128, 128], F32, tag="tr2")
                            nc.tensor.transpose(out=pst[:], in_=ctx_d[:, k, :, b],
                                                identity=ident[:])
                            nc.vector.tensor_copy(
                                out=ctxT1[:, b, 128 * k:128 * (k + 1)], in_=pst[:])
                    for m in range(4):
                        for nb in range(2):
                            ps = psm.tile([128, 64, BL], F32, tag="mid_ps")
                            tsl = slice(64 * nb, 64 * (nb + 1))
                            for k in range(4):
                                nc.tensor.matmul(
                                    out=ps[:], lhsT=swa_c[:, k, 128 * m:128 * (m + 1)],
                                    rhs=ctx_d[:, k, tsl, :], start=(k == 0), stop=False)
                            bias_mm(ps[:], sba[:, 128 * m:128 * (m + 1)], 64)
                            nc.vector.tensor_copy(out=pctx[:, m, tsl, :], in_=ps[:])

                # ---- phase 4: decoder x-parts, scan, Q, attention ----
                with tc.tile_pool(name="decx", bufs=1) as decx, \
                     tc.tile_pool(name="ps_dec", bufs=2, space="PSUM") as psd:
                    swx_d = load(decx, wx_d, [128, 4, D2])
                    swxh_d = load(decx, wxh_d, [128, 4, H])
                    xd = decx.tile([128, 4, TD, BL], F32, tag="xd")
                    xhd = decx.tile([128, 2, TD, BL], F32, tag="xhd")

                    def dxbulk(dst, wt, bias, mchunks):
                        for m in range(mchunks):
                            for nb in range(2):
                                t0c = 1 + 64 * nb
                                t1c = min(1 + 64 * (nb + 1), T)
                                nt = t1c - t0c
                                ps = psd.tile([128, 64, BL], F32, tag="bulk_d")
                                for k in range(4):
                                    nc.tensor.matmul(
                                        out=ps[:, 0:nt, :],
                                        lhsT=wt[:, k, 128 * m:128 * (m + 1)],
                                        rhs=ctx_d[:, k, t0c:t1c, :],
                                        start=(k == 0), stop=False)
                                bias_mm(ps[:, 0:nt, :], bias[:, 128 * m:128 * (m + 1)], nt)
                                nc.vector.tensor_copy(out=dst[:, m, t0c - 1:t1c - 1, :],
                                                      in_=ps[:, 0:nt, :])

                    dxbulk(xd, swx_d, sb_d, 4)
                    dxbulk(xhd, swxh_d, sbh_d, 2)

                    nc.vector.memset(hdT[:, :, 0, :], 0.0)

                    for t in range(1, T):
                        hprev = hdT[:, :, t - 1, :]
                        ps_rz = psd.tile([128, 4, BL], F32, tag="rz_d")
                        for m in range(4):
                            for k in range(2):
                                nc.tensor.matmul(
                                    out=ps_rz[:, m, :],
                                    lhsT=swh_d[:, k, 128 * m:128 * (m + 1)],
                                    rhs=hprev[:, k, :], start=(k == 0), stop=(k == 1))
                        rz = decx.tile([128, 4, BL], F32, tag="rzs_d")
                        nc.vector.tensor_add(out=rz[:], in0=ps_rz[:], in1=xd[:, :, t - 1, :])
                        rs = decx.tile([128, 4, BL], F32, tag="rs_d")
                        nc.scalar.activation(out=rs[:], in_=rz[:], func=AF.Sigmoid)
                        rh = decx.tile([128, 2, BL], F32, tag="rh_d")
                        nc.vector.tensor_mul(out=rh[:], in0=rs[:, 0:2, :], in1=hprev[:])
                        ps_hc = psd.tile([128, 2, BL], F32, tag="hc_d")
                        for m in range(2):
                            for k in range(2):
                                nc.tensor.matmul(
                                    out=ps_hc[:, m, :],
                                    lhsT=swhh_d[:, k, 128 * m:128 * (m + 1)],
                                    rhs=rh[:, k, :], start=(k == 0), stop=(k == 1))
                        hcp = decx.tile([128, 2, BL], F32, tag="hcp_d")
                        nc.vector.tensor_add(out=hcp[:], in0=ps_hc[:], in1=xhd[:, :, t - 1, :])
                        hc = decx.tile([128, 2, BL], F32, tag="hcs_d")
                        nc.scalar.activation(out=hc[:], in_=hcp[:], func=AF.Tanh)
                        tmp = decx.tile([128, 2, BL], F32, tag="tmp_d")
                        nc.vector.tensor_sub(out=tmp[:], in0=hprev[:], in1=hc[:])
                        nc.vector.tensor_mul(out=tmp[:], in0=rs[:, 2:4, :], in1=tmp[:])
                        nc.vector.tensor_add(out=hdT[:, :, t, :], in0=hc[:], in1=tmp[:])

                    # Q in chunks of 16 decoder steps
                    for ci in range(8):
                        t0c, t1c = 16 * ci, min(16 * (ci + 1), TD)
                        nt = t1c - t0c
                        ps = psd.tile([128, 4, 16, BL], F32, tag="bulk_d")
                        for m in range(4):
                            for k in range(2):
                                nc.tensor.matmul(
                                    out=ps[:, m, 0:nt, :],
                                    lhsT=swa_h[:, k, 128 * m:128 * (m + 1)],
                                    rhs=hdT[:, k, t0c:t1c, :],
                                    start=(k == 0), stop=(k == 1))
                        nc.vector.tensor_copy(out=q[:, :, t0c:t1c, :], in_=ps[:, :, 0:nt, :])

                    # attention scores
                    with tc.tile_pool(name="attn", bufs=2) as attn, \
                         tc.tile_pool(name="ps_sc", bufs=2, space="PSUM") as ps_sc:
                        for t in range(1, T):
                            sc = ps_sc.tile([128, BL], F32, tag="sc")
                            for hh in range(2):
                                u = attn.tile([128, 2, T, BL], F32, tag="u")
                                nc.vector.tensor_add(
                                    out=u[:], in0=pctx[:, 2 * hh:2 * hh + 2, :, :],
                                    in1=q[:, 2 * hh:2 * hh + 2, t - 1:t, :]
                                        .to_broadcast([128, 2, T, BL]))
                                nc.scalar.activation(out=u[:], in_=u[:], func=AF.Tanh)
                                for b in range(BL):
                                    for kk in range(2):
                                        k = 2 * hh + kk
                                        nc.tensor.matmul(
                                            out=sc[:, b:b + 1], lhsT=u[:, kk, :, b],
                                            rhs=sway[:, k:k + 1],
                                            start=(k == 0), stop=(k == 3))
                            nc.vector.tensor_copy(out=scores[:, t - 1, :], in_=sc[:])

                # ---- phase 5: softmax + wc + fusion + output ----
                with tc.tile_pool(name="fus", bufs=1) as fus, \
                     tc.tile_pool(name="ps_fus", bufs=4, space="PSUM") as psf:
                    nc.scalar.activation(out=scores[:], in_=scores[:], func=AF.Exp)
                    sums = fus.tile([1, TD, BL], F32, tag="sums")
                    TSP = [(0, 64), (64, TD)]
                    for (t0c, t1c) in TSP:
                        nt = t1c - t0c
                        ps = psf.tile([1, 64, BL], F32, tag="fusB")
                        nc.tensor.matmul(out=ps[:, 0:nt, :], lhsT=ones_col[:],
                                         rhs=scores[:, t0c:t1c, :], start=True, stop=True)
                        nc.vector.tensor_copy(out=sums[:, t0c:t1c, :], in_=ps[:, 0:nt, :])
                    nc.vector.reciprocal(out=sums[:], in_=sums[:])
                    alphas = fus.tile([128, TD, BL], F32, tag="alphas")
                    for (t0c, t1c) in TSP:
                        nt = t1c - t0c
                        ps = psf.tile([128, 64, BL], F32, tag="fusA")
                        nc.tensor.matmul(out=ps[:, 0:nt, :], lhsT=ones_row[:],
                                         rhs=sums[:, t0c:t1c, :], start=True, stop=True)
                        nc.vector.tensor_mul(out=alphas[:, t0c:t1c, :],
                                             in0=scores[:, t0c:t1c, :], in1=ps[:, 0:nt, :])

                    wcT = fus.tile([128, 4, TD, BL], F32, tag="wcT")
                    for b in range(BL):
                        for k in range(4):
                            ps = psf.tile([128, TD], F32, tag="fusB")
                            nc.tensor.matmul(out=ps[:],
                                             lhsT=ctxT1[:, b, 128 * k:128 * (k + 1)],
                                             rhs=alphas[:, :, b], start=True, stop=True)
                            nc.vector.tensor_copy(out=wcT[:, k, :, b], in_=ps[:])

                    lfc = fus.tile([128, 2, TD, BL], F32, tag="lfc")
                    for m in range(2):
                        for (t0c, t1c) in TSP:
                            nt = t1c - t0c
                            ps = psf.tile([128, 64, BL], F32, tag="fusA")
                            for k in range(4):
                                nc.tensor.matmul(
                                    out=ps[:, 0:nt, :],
                                    lhsT=swf_c[:, k, 128 * m:128 * (m + 1)],
                                    rhs=wcT[:, k, t0c:t1c, :], start=(k == 0), stop=(k == 3))
                            nc.vector.tensor_copy(out=lfc[:, m, t0c:t1c, :], in_=ps[:, 0:nt, :])

                    fw = fus.tile([128, 2, TD, BL], F32, tag="fw")
                    for m in range(2):
                        for (t0c, t1c) in TSP:
                            nt = t1c - t0c
                            ps = psf.tile([128, 64, BL], F32, tag="fusA")
                            for k in range(2):
                                nc.tensor.matmul(
                                    out=ps[:, 0:nt, :],
                                    lhsT=swf_f[:, k, 128 * m:128 * (m + 1)],
                                    rhs=lfc[:, k, t0c:t1c, :], start=(k == 0), stop=False)
                            for k in range(2):
                                nc.tensor.matmul(
                                    out=ps[:, 0:nt, :],
                                    lhsT=swf_h[:, k, 128 * m:128 * (m + 1)],
                                    rhs=hdT[:, k, t0c + 1:t1c + 1, :], start=False, stop=False)
                            bias_mm(ps[:, 0:nt, :], sbfu[:, 128 * m:128 * (m + 1)], nt)
                            nc.scalar.activation(out=fw[:, m, t0c:t1c, :], in_=ps[:, 0:nt, :],
                                                 func=AF.Sigmoid)

                    hf = fus.tile([128, 2, TD, BL], F32, tag="hf")
                    nc.vector.tensor_mul(out=hf[:], in0=lfc[:], in1=fw[:])
                    nc.vector.tensor_add(out=hf[:], in0=hf[:], in1=hdT[:, :, 1:T, :])
                    ysb = fus.tile([VY, TD, BL], F32, tag="ysb")
                    for (t0c, t1c) in TSP:
                        nt = t1c - t0c
                        ps = psf.tile([VY, 64, BL], F32, tag="fusB")
                        for k in range(2):
                            nc.tensor.matmul(out=ps[:, 0:nt, :], lhsT=swy[:, k, :],
                                             rhs=hf[:, k, t0c:t1c, :],
                                             start=(k == 0), stop=False)
                        bias_mm(ps[:, 0:nt, :], sby[:], nt)
                        nc.vector.tensor_copy(out=ysb[:, t0c:t1c, :], in_=ps[:, 0:nt, :])
                    nc.sync.dma_start(out=y_out[:], in_=ysb[:])

    nc.compile()
    return nc


def _prep_inputs(inputs, core):
    lanes = slice(core * BL, (core + 1) * BL)

    def kmaj(w, kchunks):
        return np.ascontiguousarray(
            np.asarray(w, dtype=np.float32).reshape(kchunks, 128, -1)
            .transpose(1, 0, 2))

    f32 = np.float32
    return {
        "tok": np.ascontiguousarray(np.asarray(inputs["tokens"])[:, lanes]).astype(np.int32),
        "we": np.ascontiguousarray(np.asarray(inputs["We"], dtype=f32)),
        "wx_f": kmaj(inputs["Wx_f"], 2), "wh_f": kmaj(inputs["Wh_f"], 2),
        "wxh_f": kmaj(inputs["Wxh_f"], 2), "whh_f": kmaj(inputs["Whh_f"], 2),
        "wx_b": kmaj(inputs["Wx_b"], 2), "wh_b": kmaj(inputs["Wh_b"], 2),
        "wxh_b": kmaj(inputs["Wxh_b"], 2), "whh_b": kmaj(inputs["Whh_b"], 2),
        "wx_d": kmaj(inputs["Wx_d"], 4), "wh_d": kmaj(inputs["Wh_d"], 2),
        "wxh_d": kmaj(inputs["Wxh_d"], 4), "whh_d": kmaj(inputs["Whh_d"], 2),
        "wa_c": kmaj(inputs["Wa_c"], 4), "wa_h": kmaj(inputs["Wa_h"], 2),
        "way": np.ascontiguousarray(
            np.asarray(inputs["Wa_y"], dtype=f32).reshape(4, 128).T),
        "wf_c": kmaj(inputs["Wf_c"], 4), "wf_f": kmaj(inputs["Wf_f"], 2),
        "wf_h": kmaj(inputs["Wf_h"], 2), "wy": kmaj(inputs["Wy"], 2),
        "b_f": np.asarray(inputs["b_f"], dtype=f32).reshape(1, -1),
        "bh_f": np.asarray(inputs["bh_f"], dtype=f32).reshape(1, -1),
        "b_b": np.asarray(inputs["b_b"], dtype=f32).reshape(1, -1),
        "bh_b": np.asarray(inputs["bh_b"], dtype=f32).reshape(1, -1),
        "b_d": np.asarray(inputs["b_d"], dtype=f32).reshape(1, -1),
        "bh_d": np.asarray(inputs["bh_d"], dtype=f32).reshape(1, -1),
        "ba": np.asarray(inputs["ba"], dtype=f32).reshape(1, -1),
        "bfu": np.asarray(inputs["bf"], dtype=f32).reshape(1, -1),
        "by": np.asarray(inputs["by"], dtype=f32).reshape(1, -1),
    }


def kernel(**inputs):
    global last_results
    if "prog" not in _prog_cache:
        _prog_cache["prog"] = build_program()
    nc = _prog_cache["prog"]
    in_maps = [_prep_inputs(inputs, c) for c in range(NCORE)]
    res = run_bass_kernel_spmd(nc, in_maps, list(range(NCORE)))
    last_results = res
    ys = [np.asarray(res.results[c]["y"]) for c in range(NCORE)]
    y = np.concatenate([yy.transpose(1, 2, 0) for yy in ys], axis=1)
    return np.ascontiguousarray(y).astype(np.float32)

